# revision 25
# baseline (speedup 1.0000x reference)
# BatchGAT Trainium2 Bass kernel.
#
# Reference computation (per batch b, head hd):
#   hp = h[b] @ w[hd]                      [n, 64]
#   t = tanh(hp)
#   s = t @ a_src[hd];  d = t @ a_dst[hd]  [n]
#   attn[i,j] = softmax_j(leaky_relu(s[i] + d[j], 0.2))
#   out = attn @ hp + bias_p
#
# Key identity: softmax_j is invariant to a per-i scale, so multiply
# numerator and denominator by exp(-0.2 s_i):
#   exp(leaky_relu(s_i + d_j)) * exp(-0.2 s_i)
#     = max(exp(0.8 s_i) * exp(d_j), exp(0.2 d_j))
# (selection is consistent: 0.8s + d >= 0.2d iff s + d >= 0; exp(leaky) is
# continuous at 0 so ties are exact). The second operand depends only on j —
# a per-partition scalar in a [j, i] tile — so the whole n^2 stage is ONE
# VectorE tensor_scalar op per [128, n] tile:
#   Et = (es8_bcast * ed_j) max ed2_j          (4x-mode bf16)
# No transcendental touches n^2 elements and no max-subtraction is needed
# (|s|,|d| <= ~20 keeps exp in range). The weighted sum + softmax
# denominator come from TensorE matmuls with a ones-column appended to hp,
# with hp stationary and Et the N=512 moving operand. All transposes and
# broadcasts ride on DMA engines (xbar DMA-transpose / DRAM-roundtrip
# broadcast), keeping PE/DVE/ACT for real math only.
#
# Sharding: head-parallel, one head per NeuronCore (8 heads, 8 cores); each
# core computes all 4 batches of its head.

import numpy as np
from contextlib import ExitStack

import concourse.bass as bass
import concourse.tile as tile
import concourse.mybir as mybir
from concourse import bacc
from concourse.bass_utils import run_bass_kernel_spmd

F32 = mybir.dt.float32
BF16 = mybir.dt.bfloat16
F16 = mybir.dt.float16
AF = mybir.ActivationFunctionType
ALU = mybir.AluOpType

NB = 4      # batches
NF = 64     # f_in == f_out
NH = 8      # heads == cores


def _chunks(total, size):
    out = []
    c0 = 0
    while c0 < total:
        cs = min(size, total - c0)
        out.append((c0, cs))
        c0 += cs
    return out


def build_gat_module(n=2048, nb=NB, reps=1):
    nc = bacc.Bacc("TRN2", target_bir_lowering=False)

    h_t = nc.dram_tensor("h", [nb, n, NF], F32, kind="ExternalInput")
    w_t = nc.dram_tensor("w1", [NF, NF], F32, kind="ExternalInput")
    asd_t = nc.dram_tensor("asd", [NF, 2], F32, kind="ExternalInput")
    b_t = nc.dram_tensor("biasp", [NF], F32, kind="ExternalInput")
    o_t = nc.dram_tensor("out", [nb, n, NF], F32, kind="ExternalOutput")

    NT = n // 128          # 128-row tiles
    C512 = _chunks(n, 512)
    nw = len(C512)

    with tile.TileContext(nc) as tc:
        with ExitStack() as ctx:
            consts = ctx.enter_context(tc.tile_pool(name="consts", bufs=1))
            hpool = ctx.enter_context(tc.tile_pool(name="hpool", bufs=1))
            work = ctx.enter_context(tc.tile_pool(name="work", bufs=4))
            pairbuf = ctx.enter_context(tc.tile_pool(name="pairbuf", bufs=2))
            etp = ctx.enter_context(tc.tile_pool(name="etp", bufs=5))
            outp = ctx.enter_context(tc.tile_pool(name="outp", bufs=2))
            pst = ctx.enter_context(tc.tile_pool(name="pst", bufs=3, space="PSUM"))
            pacc = ctx.enter_context(tc.tile_pool(name="pacc", bufs=1, space="PSUM"))
            drampool = ctx.enter_context(
                tc.tile_pool(name="drampool", bufs=2, space="DRAM"))

            # ---- constants ----
            # w and a_src|a_dst in bf16; w replicated at partition 0 and 64 so
            # matmuls can pair it with hT slices at either base partition.
            w_f32 = consts.tile([128, NF], F32)
            nc.sync.dma_start(out=w_f32[0:NF, :], in_=w_t[:, :])
            nc.sync.dma_start(out=w_f32[NF:128, :], in_=w_t[:, :])
            w_sb = consts.tile([128, NF], BF16)
            nc.vector.tensor_copy(w_sb, w_f32)
            asd_f32 = consts.tile([NF, 2], F32)
            nc.sync.dma_start(out=asd_f32, in_=asd_t[:, :])
            asd_sb = consts.tile([NF, 2], BF16)
            nc.vector.tensor_copy(asd_sb, asd_f32)
            bias_bc = consts.tile([128, NF], F32)
            bap = b_t[:]
            nc.gpsimd.dma_start(out=bias_bc, in_=bass.AP(
                tensor=bap.tensor, offset=bap.offset,
                ap=[[0, 128]] + list(bap.ap)))

            # ---- load h, cast to bf16, DMA-xbar-transpose:
            # hTT[half][0:64, :] = h[2*half].T, [64:128, :] = h[2*half+1].T ----
            nhalf = nb // 2
            hTT = []
            for half in range(nhalf):
                hTT_t = hpool.tile([128, n], BF16, name=f"hTT{half}")
                hTT.append(hTT_t)
            for half in range(nhalf):
                for jc in range(NT):
                    hload = work.tile([128, 128], F32, name="hload")
                    nc.sync.dma_start(
                        out=hload[:, 0:NF],
                        in_=h_t[2 * half, jc * 128:(jc + 1) * 128, :])
                    nc.sync.dma_start(
                        out=hload[:, NF:128],
                        in_=h_t[2 * half + 1, jc * 128:(jc + 1) * 128, :])
                    hcast = work.tile([128, 128], BF16, name="hcast")
                    nc.vector.tensor_copy(hcast, hload)
                    nc.sync.dma_start_transpose(
                        hTT[half][:, jc * 128:(jc + 1) * 128], hcast)

            # ---- per (batch, head-on-this-core) pair ----
            # Software-pipelined emission: stage1(b) [aux matmuls + es8
            # broadcast roundtrip], then G-part1(b-1) [psum accumulator
            # drain — split ACT/DVE], then F(b) [main matmul loop], then
            # G-part2(b-1) [output transpose/divide/store] which fills the
            # PE/DVE shadow behind the next pair. This keeps the PE busy
            # across pair boundaries (no HAM re-throttle) and hides both
            # DRAM roundtrips.
            def stage1(b):
                half, bp = b // 2, NF * (b % 2)
                hT = hTT[half][bp:bp + NF, :]    # [64, n] = h[b].T  (bf16)
                w_b = w_sb[bp:bp + NF, :]
                st = {}

                # B: hpT = w.T @ hT chunks; tanh -> T
                T_sb = pairbuf.tile([NF, n], BF16, name="T_sb")
                for icx, (c0, cs) in enumerate(C512):
                    psB = pst.tile([NF, 512], F32, tag="ps", name="psB")
                    nc.tensor.matmul(
                        psB[:, 0:cs], lhsT=w_b, rhs=hT[:, c0:c0 + cs],
                        start=True, stop=True)
                    nc.scalar.activation(
                        T_sb[:, c0:c0 + cs], psB[:, 0:cs], AF.Tanh)

                # C: es8_row [1, n] = exp(0.8 * (a_src . T)), then broadcast
                # across partitions via a chunked DRAM roundtrip — each 512
                # chunk's broadcast read starts as soon as its exp lands, so
                # the roundtrip latency overlaps the rest of the preamble.
                es8_row = pairbuf.tile([1, n], BF16, name="es8_row")
                es8_dram = drampool.tile([1, n], BF16, name="es8_dram")
                es8_bc = pairbuf.tile([128, n], BF16, name="es8_bc")
                for (c0, cs) in C512:
                    psC = pst.tile([2, 512], F32, tag="ps", name="psC")
                    nc.tensor.matmul(
                        psC[:, 0:cs], lhsT=asd_sb, rhs=T_sb[:, c0:c0 + cs],
                        start=True, stop=True)
                    nc.scalar.activation(
                        es8_row[0:1, c0:c0 + cs], psC[0:1, 0:cs], AF.Exp,
                        scale=0.8)
                    nc.sync.dma_start(
                        out=es8_dram[:, c0:c0 + cs],
                        in_=es8_row[:, c0:c0 + cs])
                    edap = es8_dram[0, c0:c0 + cs]
                    nc.sync.dma_start(out=es8_bc[:, c0:c0 + cs], in_=bass.AP(
                        tensor=edap.tensor, offset=edap.offset,
                        ap=[[0, 128]] + list(edap.ap)))
                st["es8_bc"] = es8_bc

                # A: hp_ext[:, jb, 0:64] = hp rows, col 64 = 1.0 (denominator)
                hp_ext = pairbuf.tile([128, NT, 66], BF16, name="hp_ext")
                nc.vector.memset(hp_ext, 1.0)
                for (j0, js) in _chunks(NT, 8):
                    psA = pst.tile([128, min(8, NT), NF], F32, tag="ps",
                                   name="psA")
                    for k in range(js):
                        jb = j0 + k
                        nc.tensor.matmul(
                            psA[:, k, :],
                            lhsT=hT[:, jb * 128:(jb + 1) * 128],
                            rhs=w_b, start=True, stop=True)
                    nc.scalar.copy(hp_ext[:, j0:j0 + js, 0:NF], psA[:, 0:js, :])
                st["hp_ext"] = hp_ext

                # D: d in column layout [128, NT] + exp / exp(0.2 .)
                psD = pst.tile([128, NT, 2], F32, tag="ps", name="psD")
                for jb in range(NT):
                    nc.tensor.matmul(
                        psD[:, jb, :],
                        lhsT=T_sb[:, jb * 128:(jb + 1) * 128],
                        rhs=asd_sb, start=True, stop=True)
                d_col = pairbuf.tile([128, NT], F32, name="d_col")
                nc.vector.tensor_copy(d_col, psD[:, :, 1])
                ed_col = pairbuf.tile([128, NT], F32, name="ed_col")
                ed2_col = pairbuf.tile([128, NT], F32, name="ed2_col")
                nc.scalar.activation(ed_col, d_col, AF.Exp)
                nc.scalar.activation(ed2_col, d_col, AF.Exp, scale=0.2)
                st["ed_col"], st["ed2_col"] = ed_col, ed2_col
                return st

            def stageF(st):
                # main loop: Et tile via one 2-scalar tensor_scalar op, then
                # accT[o|sum, i] += hp_ext[jb].T @ Et[jb], hp stationary, one
                # psum bank per 512-col chunk (per-bank start/stop legal).
                accT = pacc.tile([65, nw, 512], F32, name="accT")
                for jb in range(NT):
                    et = etp.tile([128, n], BF16, name="et")
                    nc.vector.tensor_scalar(
                        out=et, in0=st["es8_bc"],
                        scalar1=st["ed_col"][:, jb:jb + 1],
                        scalar2=st["ed2_col"][:, jb:jb + 1],
                        op0=ALU.mult, op1=ALU.max)
                    for icx, (c0, cs) in enumerate(C512):
                        mi = nc.tensor.matmul(
                            accT[:, icx, 0:cs],
                            lhsT=st["hp_ext"][:, jb, 0:65],
                            rhs=et[:, c0:c0 + cs],
                            start=(jb == 0), stop=(jb == NT - 1))
                        if icx > 0:
                            mi.ins.ldweights = False
                st["accT"] = accT
                return st

            def stageG1(st):
                # drain accumulator: numerators to fp16 (2^-8 scale so fp16
                # can't overflow; cancels in the division), sums to f32 and
                # through a DRAM roundtrip into column layout. Split across
                # ACT and DVE to halve the serial drain.
                accT = st["accT"]
                accT_sb = pairbuf.tile([NF, n], F16, name="accT_sb")
                sums_row = pairbuf.tile([1, n], F32, name="sums_row")
                for icx, (c0, cs) in enumerate(C512):
                    if icx % 2 == 0:
                        nc.scalar.mul(
                            accT_sb[:, c0:c0 + cs], accT[0:NF, icx, 0:cs],
                            1.0 / 256.0)
                        nc.vector.tensor_scalar_mul(
                            sums_row[0:1, c0:c0 + cs], accT[NF:65, icx, 0:cs],
                            1.0 / 256.0)
                    else:
                        nc.vector.tensor_scalar_mul(
                            accT_sb[:, c0:c0 + cs], accT[0:NF, icx, 0:cs],
                            1.0 / 256.0)
                        nc.scalar.mul(
                            sums_row[0:1, c0:c0 + cs], accT[NF:65, icx, 0:cs],
                            1.0 / 256.0)
                sums_dram = drampool.tile([1, n], F32, name="sums_dram")
                nc.sync.dma_start(out=sums_dram[:, :], in_=sums_row[:, :])
                sums_col = pairbuf.tile([128, NT], F32, name="sums_col")
                sdap = sums_dram[0, :]
                nc.sync.dma_start(out=sums_col, in_=bass.AP(
                    tensor=sdap.tensor, offset=sdap.offset,
                    ap=[[1, 128], [128, NT]]))
                r_col = pairbuf.tile([128, NT], F32, name="r_col")
                nc.vector.reciprocal(r_col, sums_col)
                st["accT_sb"], st["r_col"] = accT_sb, r_col

            def stageG2(st, b):
                # transpose numerators back to [i, o] on PE (fp16), divide +
                # bias on DVE, one batched store.
                accT_sb, r_col = st["accT_sb"], st["r_col"]
                o_full = outp.tile([128, NT, NF], F32, name="o_full")
                tro = outp.tile([128, NT, NF], F16, name="tro")
                for ic in range(NT):
                    nc.sync.dma_start_transpose(
                        tro[:, ic, :], accT_sb[:, ic * 128:(ic + 1) * 128])
                for ic in range(NT):
                    nc.vector.scalar_tensor_tensor(
                        out=o_full[:, ic, :], in0=tro[:, ic, :],
                        scalar=r_col[:, ic:ic + 1], in1=bias_bc,
                        op0=ALU.mult, op1=ALU.add)
                oap = o_t[b, :, :]
                nc.sync.dma_start(
                    out=bass.AP(tensor=oap.tensor, offset=oap.offset,
                                ap=[[NF, 128], [128 * NF, NT], [1, NF]]),
                    in_=o_full)

            pairs = [bb % nb for bb in range(nb * reps)]
            prev = None
            for b in pairs:
                st = stage1(b)
                if prev is not None:
                    stageG1(prev[0])
                stageF(st)
                if prev is not None:
                    stageG2(prev[0], prev[1])
                prev = (st, b)
            stageG1(prev[0])
            stageG2(prev[0], prev[1])

    nc.compile()
    return nc


_CACHE = {}
_last_results = None


def _get_nc(n=2048, nb=NB):
    key = (n, nb)
    if key not in _CACHE:
        _CACHE[key] = build_gat_module(n, nb)
    return _CACHE[key]


def kernel(h, adj, w, a_src, a_dst, bias_p):
    global _last_results
    h = np.ascontiguousarray(np.asarray(h, dtype=np.float32))
    w = np.asarray(w, dtype=np.float32)
    a_src = np.asarray(a_src, dtype=np.float32)
    a_dst = np.asarray(a_dst, dtype=np.float32)
    bias_p = np.ascontiguousarray(np.asarray(bias_p, dtype=np.float32))
    nb, n, _ = h.shape

    nc = _get_nc(n, nb)
    in_maps = []
    for c in range(NH):
        asd = np.ascontiguousarray(
            np.concatenate([a_src[c], a_dst[c]], axis=1).astype(np.float32))
        in_maps.append({
            "h": h,
            "w1": np.ascontiguousarray(w[c]),
            "asd": asd,
            "biasp": bias_p,
        })
    res = run_bass_kernel_spmd(nc, in_maps, core_ids=list(range(NH)))
    _last_results = res
    out = np.empty((nb, NH, n, NF), np.float32)
    for c in range(NH):
        out[:, c] = res.results[c]["out"]
    return out


# revision 27
# speedup vs baseline: 2.1698x; 2.1698x over previous
# BatchGAT Trainium2 Bass kernel.
#
# Reference computation (per batch b, head hd):
#   hp = h[b] @ w[hd]                      [n, 64]
#   t = tanh(hp)
#   s = t @ a_src[hd];  d = t @ a_dst[hd]  [n]
#   attn[i,j] = softmax_j(leaky_relu(s[i] + d[j], 0.2))
#   out = attn @ hp + bias_p
#
# Key identity: softmax_j is invariant to a per-i scale, so multiply
# numerator and denominator by exp(-0.2 s_i):
#   exp(leaky_relu(s_i + d_j)) * exp(-0.2 s_i)
#     = max(exp(0.8 s_i) * exp(d_j), exp(0.2 d_j))
# (selection is consistent: 0.8s + d >= 0.2d iff s + d >= 0; exp(leaky) is
# continuous at 0 so ties are exact). The second operand depends only on j —
# a per-partition scalar in a [j, i] tile — so the whole n^2 stage is ONE
# VectorE tensor_scalar op per [128, n] tile:
#   Et = (es8_bcast * ed_j) max ed2_j          (4x-mode bf16)
# No transcendental touches n^2 elements and no max-subtraction is needed
# (|s|,|d| <= ~20 keeps exp in range). The weighted sum + softmax
# denominator come from TensorE matmuls with a ones-column appended to hp,
# with hp stationary and Et the N=512 moving operand. All transposes and
# broadcasts ride on DMA engines (xbar DMA-transpose / DRAM-roundtrip
# broadcast), keeping PE/DVE/ACT for real math only.
#
# Sharding: head-parallel, one head per NeuronCore (8 heads, 8 cores); each
# core computes all 4 batches of its head.

import numpy as np
from contextlib import ExitStack

import concourse.bass as bass
import concourse.tile as tile
import concourse.mybir as mybir
from concourse import bacc
from concourse.bass_utils import run_bass_kernel_spmd

F32 = mybir.dt.float32
BF16 = mybir.dt.bfloat16
F16 = mybir.dt.float16
AF = mybir.ActivationFunctionType
ALU = mybir.AluOpType

NB = 4      # batches
NF = 64     # f_in == f_out
NH = 8      # heads == cores


def _chunks(total, size):
    out = []
    c0 = 0
    while c0 < total:
        cs = min(size, total - c0)
        out.append((c0, cs))
        c0 += cs
    return out


def build_gat_module(n=2048, nb=NB, reps=1):
    nc = bacc.Bacc("TRN2", target_bir_lowering=False)

    h_t = nc.dram_tensor("h", [nb, n, NF], F32, kind="ExternalInput")
    w_t = nc.dram_tensor("w1", [NF, NF], F32, kind="ExternalInput")
    asd_t = nc.dram_tensor("asd", [NF, 2], F32, kind="ExternalInput")
    b_t = nc.dram_tensor("biasp", [NF], F32, kind="ExternalInput")
    o_t = nc.dram_tensor("out", [nb, n, NF], F32, kind="ExternalOutput")

    NT = n // 128          # 128-row tiles
    C512 = _chunks(n, 512)
    nw = len(C512)

    with tile.TileContext(nc) as tc:
        with ExitStack() as ctx:
            consts = ctx.enter_context(tc.tile_pool(name="consts", bufs=1))
            hpool = ctx.enter_context(tc.tile_pool(name="hpool", bufs=1))
            work = ctx.enter_context(tc.tile_pool(name="work", bufs=4))
            pairbuf = ctx.enter_context(tc.tile_pool(name="pairbuf", bufs=2))
            etp = ctx.enter_context(tc.tile_pool(name="etp", bufs=5))
            outp = ctx.enter_context(tc.tile_pool(name="outp", bufs=2))
            pst = ctx.enter_context(tc.tile_pool(name="pst", bufs=3, space="PSUM"))
            pacc = ctx.enter_context(tc.tile_pool(name="pacc", bufs=1, space="PSUM"))
            drampool = ctx.enter_context(
                tc.tile_pool(name="drampool", bufs=2, space="DRAM"))

            # ---- constants ----
            ident_bf = consts.tile([128, 128], BF16)
            from concourse.masks import make_identity
            make_identity(nc, ident_bf)
            ident_f16 = consts.tile([128, 128], F16)
            make_identity(nc, ident_f16)
            # w and a_src|a_dst in bf16; w replicated at partition 0 and 64 so
            # matmuls can pair it with hT slices at either base partition.
            w_f32 = consts.tile([128, NF], F32)
            nc.sync.dma_start(out=w_f32[0:NF, :], in_=w_t[:, :])
            nc.sync.dma_start(out=w_f32[NF:128, :], in_=w_t[:, :])
            w_sb = consts.tile([128, NF], BF16)
            nc.vector.tensor_copy(w_sb, w_f32)
            asd_f32 = consts.tile([NF, 2], F32)
            nc.sync.dma_start(out=asd_f32, in_=asd_t[:, :])
            asd_sb = consts.tile([NF, 2], BF16)
            nc.vector.tensor_copy(asd_sb, asd_f32)
            bias_bc = consts.tile([128, NF], F32)
            bap = b_t[:]
            nc.gpsimd.dma_start(out=bias_bc, in_=bass.AP(
                tensor=bap.tensor, offset=bap.offset,
                ap=[[0, 128]] + list(bap.ap)))

            # ---- load h, cast to bf16, DMA-xbar-transpose:
            # hTT[half][0:64, :] = h[2*half].T, [64:128, :] = h[2*half+1].T ----
            nhalf = nb // 2
            hTT = []
            for half in range(nhalf):
                hTT_t = hpool.tile([128, n], BF16, name=f"hTT{half}")
                hTT.append(hTT_t)
            def preamble(half):
                for jc in range(NT):
                    hload = work.tile([128, 128], F32, name="hload")
                    nc.sync.dma_start(
                        out=hload[:, 0:NF],
                        in_=h_t[2 * half, jc * 128:(jc + 1) * 128, :])
                    nc.sync.dma_start(
                        out=hload[:, NF:128],
                        in_=h_t[2 * half + 1, jc * 128:(jc + 1) * 128, :])
                    hcast = work.tile([128, 128], BF16, name="hcast")
                    nc.vector.tensor_copy(hcast, hload)
                    pstr = pst.tile([128, 128], BF16, tag="ps", name="pstr")
                    nc.tensor.transpose(pstr, hcast, ident_bf)
                    dst = hTT[half][:, jc * 128:(jc + 1) * 128]
                    if jc % 2 == 0:
                        nc.vector.tensor_copy(dst, pstr)
                    else:
                        nc.scalar.copy(dst, pstr)

            preamble(0)

            # ---- per (batch, head-on-this-core) pair ----
            # Software-pipelined emission: stage1(b) [aux matmuls + es8
            # broadcast roundtrip], then G-part1(b-1) [psum accumulator
            # drain — split ACT/DVE], then F(b) [main matmul loop], then
            # G-part2(b-1) [output transpose/divide/store] which fills the
            # PE/DVE shadow behind the next pair. This keeps the PE busy
            # across pair boundaries (no HAM re-throttle) and hides both
            # DRAM roundtrips.
            def stage1(b):
                half, bp = b // 2, NF * (b % 2)
                hT = hTT[half][bp:bp + NF, :]    # [64, n] = h[b].T  (bf16)
                w_b = w_sb[bp:bp + NF, :]
                st = {}

                # B: hpT = w.T @ hT chunks; tanh -> T
                T_sb = pairbuf.tile([NF, n], BF16, name="T_sb")
                for icx, (c0, cs) in enumerate(C512):
                    psB = pst.tile([NF, 512], F32, tag="ps", name="psB")
                    nc.tensor.matmul(
                        psB[:, 0:cs], lhsT=w_b, rhs=hT[:, c0:c0 + cs],
                        start=True, stop=True)
                    nc.scalar.activation(
                        T_sb[:, c0:c0 + cs], psB[:, 0:cs], AF.Tanh)

                # C: es8_row [1, n] = exp(0.8 * (a_src . T)), then broadcast
                # across partitions via a chunked DRAM roundtrip — each 512
                # chunk's broadcast read starts as soon as its exp lands, so
                # the roundtrip latency overlaps the rest of the preamble.
                es8_row = pairbuf.tile([1, n], BF16, name="es8_row")
                es8_dram = drampool.tile([1, n], BF16, name="es8_dram")
                es8_bc = pairbuf.tile([128, n], BF16, name="es8_bc")
                for (c0, cs) in C512:
                    psC = pst.tile([2, 512], F32, tag="ps", name="psC")
                    nc.tensor.matmul(
                        psC[:, 0:cs], lhsT=asd_sb, rhs=T_sb[:, c0:c0 + cs],
                        start=True, stop=True)
                    nc.scalar.activation(
                        es8_row[0:1, c0:c0 + cs], psC[0:1, 0:cs], AF.Exp,
                        scale=0.8)
                    nc.sync.dma_start(
                        out=es8_dram[:, c0:c0 + cs],
                        in_=es8_row[:, c0:c0 + cs])
                    edap = es8_dram[0, c0:c0 + cs]
                    nc.sync.dma_start(out=es8_bc[:, c0:c0 + cs], in_=bass.AP(
                        tensor=edap.tensor, offset=edap.offset,
                        ap=[[0, 128]] + list(edap.ap)))
                st["es8_bc"] = es8_bc

                # A: hp_ext[:, jb, 0:64] = hp rows, col 64 = 1.0 (denominator)
                hp_ext = pairbuf.tile([128, NT, 66], BF16, name="hp_ext")
                nc.vector.memset(hp_ext, 1.0)
                for (j0, js) in _chunks(NT, 8):
                    psA = pst.tile([128, min(8, NT), NF], F32, tag="ps",
                                   name="psA")
                    for k in range(js):
                        jb = j0 + k
                        nc.tensor.matmul(
                            psA[:, k, :],
                            lhsT=hT[:, jb * 128:(jb + 1) * 128],
                            rhs=w_b, start=True, stop=True)
                    nc.scalar.copy(hp_ext[:, j0:j0 + js, 0:NF], psA[:, 0:js, :])
                st["hp_ext"] = hp_ext

                # D: d in column layout [128, NT] + exp / exp(0.2 .)
                psD = pst.tile([128, NT, 2], F32, tag="ps", name="psD")
                for jb in range(NT):
                    nc.tensor.matmul(
                        psD[:, jb, :],
                        lhsT=T_sb[:, jb * 128:(jb + 1) * 128],
                        rhs=asd_sb, start=True, stop=True)
                d_col = pairbuf.tile([128, NT], F32, name="d_col")
                nc.vector.tensor_copy(d_col, psD[:, :, 1])
                ed_col = pairbuf.tile([128, NT], F32, name="ed_col")
                ed2_col = pairbuf.tile([128, NT], F32, name="ed2_col")
                nc.scalar.activation(ed_col, d_col, AF.Exp)
                nc.scalar.activation(ed2_col, d_col, AF.Exp, scale=0.2)
                st["ed_col"], st["ed2_col"] = ed_col, ed2_col
                return st

            def stageF(st):
                # main loop: Et tile via one 2-scalar tensor_scalar op, then
                # accT[o|sum, i] += hp_ext[jb].T @ Et[jb], hp stationary, one
                # psum bank per 512-col chunk (per-bank start/stop legal).
                accT = pacc.tile([65, nw, 512], F32, name="accT")
                for jb in range(NT):
                    et = etp.tile([128, n], BF16, name="et")
                    nc.vector.tensor_scalar(
                        out=et, in0=st["es8_bc"],
                        scalar1=st["ed_col"][:, jb:jb + 1],
                        scalar2=st["ed2_col"][:, jb:jb + 1],
                        op0=ALU.mult, op1=ALU.max)
                    for icx, (c0, cs) in enumerate(C512):
                        mi = nc.tensor.matmul(
                            accT[:, icx, 0:cs],
                            lhsT=st["hp_ext"][:, jb, 0:65],
                            rhs=et[:, c0:c0 + cs],
                            start=(jb == 0), stop=(jb == NT - 1))
                        if icx > 0:
                            mi.ins.ldweights = False
                st["accT"] = accT
                return st

            def stageG1(st):
                # drain accumulator: numerators to fp16 (2^-8 scale so fp16
                # can't overflow; cancels in the division), sums to f32 and
                # through a DRAM roundtrip into column layout. Split across
                # ACT and DVE to halve the serial drain.
                accT = st["accT"]
                accT_sb = pairbuf.tile([65, n], F16, name="accT_sb")
                for icx, (c0, cs) in enumerate(C512):
                    eng = nc.scalar if icx % 2 == 0 else nc.vector
                    if icx % 2 == 0:
                        nc.scalar.mul(
                            accT_sb[:, c0:c0 + cs], accT[:, icx, 0:cs],
                            1.0 / 256.0)
                    else:
                        nc.vector.tensor_scalar_mul(
                            accT_sb[:, c0:c0 + cs], accT[:, icx, 0:cs],
                            1.0 / 256.0)
                st["accT_sb"] = accT_sb

            def stageG2(st, b):
                # transpose numerators back to [i, o] on PE (fp16), divide +
                # bias on DVE, one batched store.
                accT_sb = st["accT_sb"]
                o_full = outp.tile([128, NT, NF], F32, name="o_full")
                for ic in range(NT):
                    trp = pst.tile([128, 65], F16, tag="ps", name="trp")
                    nc.tensor.transpose(
                        trp, accT_sb[:, ic * 128:(ic + 1) * 128],
                        ident_f16[0:65, 0:65])
                    r = outp.tile([128, 1], F32, name="r")
                    nc.vector.reciprocal(r, trp[:, 64:65])
                    nc.vector.scalar_tensor_tensor(
                        out=o_full[:, ic, :], in0=trp[:, 0:NF],
                        scalar=r, in1=bias_bc,
                        op0=ALU.mult, op1=ALU.add)
                oap = o_t[b, :, :]
                nc.sync.dma_start(
                    out=bass.AP(tensor=oap.tensor, offset=oap.offset,
                                ap=[[NF, 128], [128 * NF, NT], [1, NF]]),
                    in_=o_full)

            pairs = [bb % nb for bb in range(nb * reps)]
            prev = None
            first_st = stage1(pairs[0])
            for half in range(1, nhalf):
                preamble(half)
            for b in pairs:
                st = first_st if first_st is not None else stage1(b)
                first_st = None
                if prev is not None:
                    stageG1(prev[0])
                stageF(st)
                if prev is not None:
                    stageG2(prev[0], prev[1])
                prev = (st, b)
            stageG1(prev[0])
            stageG2(prev[0], prev[1])

    nc.compile()
    return nc


_CACHE = {}
_last_results = None


def _get_nc(n=2048, nb=NB):
    key = (n, nb)
    if key not in _CACHE:
        _CACHE[key] = build_gat_module(n, nb)
    return _CACHE[key]


def kernel(h, adj, w, a_src, a_dst, bias_p):
    global _last_results
    h = np.ascontiguousarray(np.asarray(h, dtype=np.float32))
    w = np.asarray(w, dtype=np.float32)
    a_src = np.asarray(a_src, dtype=np.float32)
    a_dst = np.asarray(a_dst, dtype=np.float32)
    bias_p = np.ascontiguousarray(np.asarray(bias_p, dtype=np.float32))
    nb, n, _ = h.shape

    nc = _get_nc(n, nb)
    in_maps = []
    for c in range(NH):
        asd = np.ascontiguousarray(
            np.concatenate([a_src[c], a_dst[c]], axis=1).astype(np.float32))
        in_maps.append({
            "h": h,
            "w1": np.ascontiguousarray(w[c]),
            "asd": asd,
            "biasp": bias_p,
        })
    res = run_bass_kernel_spmd(nc, in_maps, core_ids=list(range(NH)))
    _last_results = res
    out = np.empty((nb, NH, n, NF), np.float32)
    for c in range(NH):
        out[:, c] = res.results[c]["out"]
    return out
